# revision 1
# baseline (speedup 1.0000x reference)
"""Trainium2 Bass kernel for the LIF dense layer (spike output only).

The reference computes
    P_n   = quant8(alpha*P + Q)            (grid 1/128, round-half-even)
    U     = P_n @ quant8(W) + quant8(b) - S
    S_n   = (U > 0.4)
``input_t`` and ``R`` never influence the output (Q_n/U_q are dead,
gamma == 0), so they are never loaded.

All quantized operands are 8-bit integers scaled by 1/128, hence exactly
representable in bf16, and every partial matmul sum is a multiple of 2^-14
below 2^24 -> bf16 matmul with fp32 PSUM accumulation is bit-exact vs the
fp32 reference einsum.  Rounding uses the fp32 magic-number trick
(x + 1.5*2^16) - 1.5*2^16 == round-to-nearest-even onto the 1/128 grid.

Sharding: pure data parallel over the batch dim, 4096 rows per core on 8
NeuronCores; the [512,512] weights / bias are quantized host-side (exact
replication of the reference quantizer) and replicated.
"""

import sys

import numpy as np

sys.path.insert(0, "/opt/trn_rl_repo")

import ml_dtypes

B, IN, OUT = 32768, 512, 512
NCORES = 8
BL = B // NCORES            # rows per core
PART = 128                  # SBUF partitions
KCH = IN // PART            # contraction chunks of 128
G = 2                       # 128-row tiles per pipeline step
# exp(-dt/tau_mem) as computed by XLA fp32 (1 ulp above numpy's expf)
ALPHA = float(np.array(1062312023, np.uint32).view(np.float32))
MAGIC = 98304.0             # 1.5*2^16: fp32 +/- rounds to multiples of 2^-7
QMAX = 127.0 / 128.0
THR = 0.4


def build_nc(bl=BL, g=G, enable_asserts=False):
    import concourse.bass as bass
    import concourse.bacc as bacc
    import concourse.mybir as mybir
    from concourse import tile

    OP = mybir.AluOpType
    AF = mybir.ActivationFunctionType
    dt = mybir.dt
    ts = bass.ts

    ntiles = bl // PART
    assert ntiles % g == 0
    nsuper = ntiles // g

    # Bacc (not plain Bass): its compile() splits multi-sem waits into
    # event semaphores -- TRN2 allows one wait per instruction.
    nc = bacc.Bacc(
        "TRN2",
        target_bir_lowering=False,
        debug=False,
        enable_asserts=enable_asserts,
        num_devices=NCORES,
    )
    p_d = nc.dram_tensor("p", [bl, IN], dt.float32, kind="ExternalInput").ap()
    q_d = nc.dram_tensor("q", [bl, IN], dt.float32, kind="ExternalInput").ap()
    s_d = nc.dram_tensor("s", [bl, OUT], dt.float32, kind="ExternalInput").ap()
    w_d = nc.dram_tensor("w", [IN, OUT], dt.bfloat16, kind="ExternalInput").ap()
    b_d = nc.dram_tensor("bq", [1, OUT], dt.bfloat16, kind="ExternalInput").ap()
    o_d = nc.dram_tensor("o", [bl, OUT], dt.float32, kind="ExternalOutput").ap()

    # partition-major views: one DMA moves [128, g, 512] per super-tile
    pv = p_d.rearrange("(n p) i -> p n i", p=PART)
    qv = q_d.rearrange("(n p) i -> p n i", p=PART)
    sv = s_d.rearrange("(n p) i -> p n i", p=PART)
    ov = o_d.rearrange("(n p) i -> p n i", p=PART)
    wv = w_d.rearrange("(k p) o -> p k o", p=PART)

    with tile.TileContext(nc) as tc:
        with (
            tc.tile_pool(name="const", bufs=1) as cpool,
            tc.tile_pool(name="io", bufs=3) as iop,
            tc.tile_pool(name="work", bufs=2) as wkp,
            tc.tile_pool(name="ps", bufs=2, space="PSUM") as psp,
        ):
            w_sb = cpool.tile([PART, KCH, OUT], dt.bfloat16)
            nc.sync.dma_start(out=w_sb[:], in_=wv[:])
            bq_sb = cpool.tile([1, OUT], dt.bfloat16)
            nc.sync.dma_start(out=bq_sb[:], in_=b_d[:])
            ones_sb = cpool.tile([1, PART], dt.bfloat16)
            nc.vector.memset(ones_sb[:], 1.0)
            magic_p = cpool.tile([PART, 1], dt.float32)
            nc.vector.memset(magic_p[:], MAGIC)
            magic_n = cpool.tile([PART, 1], dt.float32)
            nc.vector.memset(magic_n[:], -MAGIC)

            for si in range(nsuper):
                p_t = iop.tile([PART, g, IN], dt.float32, tag="p")
                q_t = iop.tile([PART, g, IN], dt.float32, tag="q")
                s_t = iop.tile([PART, g, OUT], dt.float32, tag="s")
                tsl = slice(si * g, (si + 1) * g)
                nc.sync.dma_start(out=p_t[:], in_=pv[:, tsl, :])
                nc.sync.dma_start(out=q_t[:], in_=qv[:, tsl, :])
                nc.sync.dma_start(out=s_t[:], in_=sv[:, tsl, :])

                # x = alpha*P + Q  (fp32); scale on ACT, add on DVE
                x_t = wkp.tile([PART, g, IN], dt.float32, tag="x")
                nc.scalar.activation(x_t[:], p_t[:], AF.Copy, scale=ALPHA)
                nc.vector.tensor_add(x_t[:], x_t[:], q_t[:])
                # round-half-even onto the 1/128 grid (ACT engine)
                nc.scalar.activation(x_t[:], x_t[:], AF.Identity, bias=magic_p[:])
                nc.scalar.activation(x_t[:], x_t[:], AF.Identity, bias=magic_n[:])
                # saturate to +/-127/128 and narrow to bf16 (exact)
                q8_t = wkp.tile([PART, g, IN], dt.bfloat16, tag="q8")
                nc.vector.tensor_scalar(
                    out=q8_t[:], in0=x_t[:], scalar1=QMAX, scalar2=-QMAX,
                    op0=OP.min, op1=OP.max,
                )
                # xbar transpose: contract dim onto partitions
                q8T_t = wkp.tile([PART, g, KCH, PART], dt.bfloat16, tag="q8T")
                for j in range(g):
                    for k in range(KCH):
                        nc.scalar.dma_start_transpose(
                            out=q8T_t[:, j, k, :],
                            in_=q8_t[:, j, ts(k, PART)],
                        )
                u_ps = psp.tile([PART, g, OUT], dt.float32, tag="u")
                for j in range(g):
                    for k in range(KCH):
                        nc.tensor.matmul(
                            u_ps[:, j, :],
                            lhsT=q8T_t[:, j, k, :],
                            rhs=w_sb[:, k, :],
                            start=(k == 0),
                            stop=False,
                        )
                    # bias as a K=1 accumulation: ones.T @ bq
                    nc.tensor.matmul(
                        u_ps[:, j, :], lhsT=ones_sb[:], rhs=bq_sb[:],
                        start=False, stop=True,
                    )
                # spike = (U - S) > 0.4  (exact: U-S is on the 2^-14 grid)
                sp_t = iop.tile([PART, g, OUT], dt.float32, tag="sp")
                nc.vector.tensor_sub(sp_t[:], u_ps[:], s_t[:])
                nc.vector.tensor_scalar(
                    out=sp_t[:], in0=sp_t[:], scalar1=THR, scalar2=None,
                    op0=OP.is_gt,
                )
                nc.sync.dma_start(out=ov[:, tsl, :], in_=sp_t[:])
    nc.finalize()  # Bacc.compile(): splits multi-sem waits (TRN2 1-wait rule)
    return nc


def _quant_host(x):
    """Exact replica of the reference quant_ste forward pass (fp32)."""
    x = np.asarray(x, np.float32)
    d = np.float32(1.0) / np.float32(128.0)
    y = np.clip(x, np.float32(-1.0) + d, np.float32(1.0) - d)
    y = y * np.float32(128.0)
    y = np.round(y)  # round-half-even, same as jnp.round
    return (y / np.float32(128.0)).astype(np.float32)


_cache = {}


def kernel(**inputs):
    from concourse.bass_utils import run_bass_kernel_spmd

    P = np.ascontiguousarray(np.asarray(inputs["P"], np.float32))
    Q = np.ascontiguousarray(np.asarray(inputs["Q"], np.float32))
    S = np.ascontiguousarray(np.asarray(inputs["S"], np.float32))
    W = np.asarray(inputs["weights"], np.float32)
    bias = np.asarray(inputs["bias"], np.float32)

    wq = _quant_host(W).astype(ml_dtypes.bfloat16)
    bq = _quant_host(bias).reshape(1, OUT).astype(ml_dtypes.bfloat16)

    if "nc" not in _cache:
        _cache["nc"] = build_nc()
    nc = _cache["nc"]

    in_maps = []
    for c in range(NCORES):
        sl = slice(c * BL, (c + 1) * BL)
        in_maps.append({"p": P[sl], "q": Q[sl], "s": S[sl], "w": wq, "bq": bq})
    res = run_bass_kernel_spmd(nc, in_maps, list(range(NCORES)))
    _cache["last"] = res  # exec_time_ns etc. when tracing is enabled
    out = np.concatenate([res.results[c]["o"] for c in range(NCORES)], axis=0)
    return np.ascontiguousarray(out.astype(np.float32))



# revision 2
# speedup vs baseline: 3.0824x; 3.0824x over previous
"""Trainium2 Bass kernel for the LIF dense layer (spike output only).

The reference computes
    P_n   = quant8(alpha*P + Q)            (grid 1/128, round-half-even)
    U     = P_n @ quant8(W) + quant8(b) - S
    S_n   = (U > 0.4)
``input_t`` and ``R`` never influence the output (Q_n/U_q are dead,
gamma == 0), so they are never loaded.

All quantized operands are 8-bit integers scaled by 1/128, hence exactly
representable in bf16, and every partial matmul sum is a multiple of 2^-14
below 2^24 -> bf16 matmul with fp32 PSUM accumulation is bit-exact vs the
fp32 reference einsum.  Rounding uses the fp32 magic-number trick
(x + 1.5*2^16) - 1.5*2^16 == round-to-nearest-even onto the 1/128 grid.
The epilogue compares (U - 0.4) > S in one DVE op: U is exact on the
2^-14 grid, so the fp32 subtract of 0.4f never flips the comparison
(error <= 2^-19 vs a >= 2.4e-5 gap to the threshold).

Activation tiles are transposed for the matmul on the TensorEngine
(identity-matmul transpose into PSUM, ACT copies back to SBUF) -- the DMA
xbar transpose path serializes on the ACT sequencer at ~1.2us per issue
and dominated the old kernel.

Sharding: pure data parallel over the batch dim, 4096 rows per core on 8
NeuronCores; the [512,512] weights / bias are quantized host-side (exact
replication of the reference quantizer) and replicated.
"""

import sys

import numpy as np

sys.path.insert(0, "/opt/trn_rl_repo")

import ml_dtypes

B, IN, OUT = 32768, 512, 512
NCORES = 8
BL = B // NCORES            # rows per core
PART = 128                  # SBUF partitions
KCH = IN // PART            # contraction chunks of 128
G = 4                       # 128-row tiles per pipeline step (1 MiB DMAs)
# exp(-dt/tau_mem) as computed by XLA fp32 (1 ulp above numpy's expf)
ALPHA = float(np.array(1062312023, np.uint32).view(np.float32))
MAGIC = 98304.0             # 1.5*2^16: fp32 +/- rounds to multiples of 2^-7
QMAX = 127.0 / 128.0
THR = 0.4
A_BITS = [1, 4]


def build_nc(bl=BL, g=G, enable_asserts=False):
    import concourse.bass as bass
    import concourse.bacc as bacc
    import concourse.mybir as mybir
    from concourse import tile
    from concourse.masks import make_identity

    OP = mybir.AluOpType
    AF = mybir.ActivationFunctionType
    dt = mybir.dt
    ts = bass.ts

    ntiles = bl // PART
    assert ntiles % g == 0
    nsuper = ntiles // g

    # Bacc (not plain Bass): its compile() splits multi-sem waits into
    # event semaphores -- TRN2 allows one wait per instruction.
    nc = bacc.Bacc(
        "TRN2",
        target_bir_lowering=False,
        debug=False,
        enable_asserts=enable_asserts,
        num_devices=NCORES,
    )
    p_d = nc.dram_tensor("p", [bl, IN], dt.float32, kind="ExternalInput").ap()
    q_d = nc.dram_tensor("q", [bl, IN], dt.float32, kind="ExternalInput").ap()
    s_d = nc.dram_tensor("s", [bl, OUT], dt.float32, kind="ExternalInput").ap()
    w_d = nc.dram_tensor("w", [IN, OUT], dt.bfloat16, kind="ExternalInput").ap()
    b_d = nc.dram_tensor("bq", [1, OUT], dt.bfloat16, kind="ExternalInput").ap()
    o_d = nc.dram_tensor("o", [bl, OUT], dt.float32, kind="ExternalOutput").ap()

    # partition-major views: one DMA moves [128, g, 512] per super-tile
    pv = p_d.rearrange("(n p) i -> p n i", p=PART)
    qv = q_d.rearrange("(n p) i -> p n i", p=PART)
    sv = s_d.rearrange("(n p) i -> p n i", p=PART)
    ov = o_d.rearrange("(n p) i -> p n i", p=PART)
    wv = w_d.rearrange("(k p) o -> p k o", p=PART)

    with tile.TileContext(nc) as tc:
        with (
            tc.tile_pool(name="const", bufs=1) as cpool,
            tc.tile_pool(name="io", bufs=3) as iop,
            tc.tile_pool(name="work", bufs=2) as wkp,
            tc.tile_pool(name="pst", bufs=2, space="PSUM") as psp_t,
            tc.tile_pool(name="psu", bufs=2, space="PSUM") as psp_u,
        ):
            w_sb = cpool.tile([PART, KCH, OUT], dt.bfloat16)
            nc.sync.dma_start(out=w_sb[:], in_=wv[:])
            bq_sb = cpool.tile([1, OUT], dt.bfloat16)
            nc.sync.dma_start(out=bq_sb[:], in_=b_d[:])
            ones_sb = cpool.tile([1, PART], dt.bfloat16)
            nc.vector.memset(ones_sb[:], 1.0)
            ident = cpool.tile([PART, PART], dt.bfloat16)
            make_identity(nc, ident[:])

            for si in range(nsuper):
                p_t = iop.tile([PART, g, IN], dt.float32, tag="p")
                q_t = iop.tile([PART, g, IN], dt.float32, tag="q")
                s_t = iop.tile([PART, g, OUT], dt.float32, tag="s")
                tsl = slice(si * g, (si + 1) * g)
                nc.sync.dma_start(out=p_t[:], in_=pv[:, tsl, :])
                nc.sync.dma_start(out=q_t[:], in_=qv[:, tsl, :])
                nc.sync.dma_start(out=s_t[:], in_=sv[:, tsl, :])

                # x = alpha*P + Q (one DVE pass; each ALU slice rounds fp32)
                x_t = wkp.tile([PART, g, IN], dt.float32, tag="x")
                nc.vector.scalar_tensor_tensor(
                    out=x_t[:], in0=p_t[:], scalar=ALPHA, in1=q_t[:],
                    op0=OP.mult, op1=OP.add,
                )
                # round-half-even onto the 1/128 grid (magic add/sub)
                nc.vector.tensor_scalar(
                    out=x_t[:], in0=x_t[:], scalar1=MAGIC, scalar2=MAGIC,
                    op0=OP.add, op1=OP.subtract,
                )
                # saturate to +/-127/128 and narrow to bf16 (exact)
                q8_t = wkp.tile([PART, g, IN], dt.bfloat16, tag="q8")
                nc.vector.tensor_scalar(
                    out=q8_t[:], in0=x_t[:], scalar1=QMAX, scalar2=-QMAX,
                    op0=OP.min, op1=OP.max,
                )
                sp_t = iop.tile([PART, g, OUT], dt.float32, tag="sp")
                for j in range(g):
                    # PE transpose: contract dim onto partitions (bf16)
                    tp = psp_t.tile([PART, KCH, PART], dt.bfloat16, tag="tp")
                    for k in range(KCH):
                        nc.tensor.transpose(
                            tp[:, k, :], q8_t[:, j, ts(k, PART)], ident[:]
                        )
                    q8T = wkp.tile([PART, KCH, PART], dt.bfloat16, tag="q8T")
                    nc.scalar.activation(q8T[:], tp[:], AF.Copy)
                    up = psp_u.tile([PART, OUT], dt.float32, tag="up")
                    for k in range(KCH):
                        nc.tensor.matmul(
                            up[:],
                            lhsT=q8T[:, k, :],
                            rhs=w_sb[:, k, :],
                            start=(k == 0),
                            stop=False,
                        )
                    # bias as a K=1 accumulation: ones.T @ bq
                    nc.tensor.matmul(
                        up[:], lhsT=ones_sb[:], rhs=bq_sb[:],
                        start=False, stop=True,
                    )
                    # spike = (U - 0.4) > S in one DVE op
                    nc.vector.scalar_tensor_tensor(
                        out=sp_t[:, j, :], in0=up[:], scalar=THR,
                        in1=s_t[:, j, :], op0=OP.subtract, op1=OP.is_gt,
                    )
                nc.scalar.dma_start(out=ov[:, tsl, :], in_=sp_t[:])
    nc.finalize()  # Bacc.compile(): splits multi-sem waits (TRN2 1-wait rule)
    return nc


def _quant_host(x):
    """Exact replica of the reference quant_ste forward pass (fp32)."""
    x = np.asarray(x, np.float32)
    d = np.float32(1.0) / np.float32(128.0)
    y = np.clip(x, np.float32(-1.0) + d, np.float32(1.0) - d)
    y = y * np.float32(128.0)
    y = np.round(y)  # round-half-even, same as jnp.round
    return (y / np.float32(128.0)).astype(np.float32)


_cache = {}


def kernel(**inputs):
    from concourse.bass_utils import run_bass_kernel_spmd

    P = np.ascontiguousarray(np.asarray(inputs["P"], np.float32))
    Q = np.ascontiguousarray(np.asarray(inputs["Q"], np.float32))
    S = np.ascontiguousarray(np.asarray(inputs["S"], np.float32))
    W = np.asarray(inputs["weights"], np.float32)
    bias = np.asarray(inputs["bias"], np.float32)

    wq = _quant_host(W).astype(ml_dtypes.bfloat16)
    bq = _quant_host(bias).reshape(1, OUT).astype(ml_dtypes.bfloat16)

    if "nc" not in _cache:
        _cache["nc"] = build_nc()
    nc = _cache["nc"]

    in_maps = []
    for c in range(NCORES):
        sl = slice(c * BL, (c + 1) * BL)
        in_maps.append({"p": P[sl], "q": Q[sl], "s": S[sl], "w": wq, "bq": bq})
    res = run_bass_kernel_spmd(nc, in_maps, list(range(NCORES)))
    _cache["last"] = res  # exec_time_ns etc. when tracing is enabled
    out = np.concatenate([res.results[c]["o"] for c in range(NCORES)], axis=0)
    return np.ascontiguousarray(out.astype(np.float32))
